# revision 1
# baseline (speedup 1.0000x reference)
"""Contrastive cosine-similarity MSE loss kernel for Trainium2 (8 cores).

Math (reference): scores_n = <a_n, b_n> / (||a_n|| * ||b_n||);
loss = mean((scores - labels)^2) over N=8192 rows, D=1024.

The kernel is at the compute/memory ridge: per core it reads 4.2 MB
(fp16) in ~12 us, and needs 24 row-stat reductions (8 blocks x
{dot, ||a||^2, ||b||^2}) which at the engines' 1x reduce rate would
take ~14 us serialized. Structure:
  - ScalarE: 9 direct square-accumulate stats (na_0..7, nb_7).
  - VectorE: 15 stats as plain fp16 tensor_tensor products (a*b, b*b),
    which hit the DVE's 2x_1P mode (~600 ns per [128,1024] block vs
    ~1136 ns for a 1x fused reduce).
  - TensorE: folds each product [128,1024] -> PSUM [128,128] with 8
    identity-stationary accumulating matmuls (psum += chunk). At the
    full 2.4 GHz p-state these run at ~109 ns each with the implicit
    LDWEIGHTS hidden; warmup + filler matmuls keep the PE p-state from
    decaying during DMA gaps (a cold/mid-state matmul is 2-3x slower).
  - VectorE: one segmented tensor_reduce per block PAIR finishes
    [128,4,128] -> [128,4] out of PSUM (~660 ns per 2 blocks).
  - The cosine+MSE tail runs incrementally per block-pair so only the
    last pair's ~0.5 us sits on the critical path.
Embeddings are downcast to fp16 on the host (cosine is scale-invariant
to first order; measured end-to-end loss error ~1e-7). All reductions
accumulate in fp32.

Sharding: data-parallel over rows; core c handles rows
[c*1024, (c+1)*1024). Tiles are [128 partitions x 2048] fp16 where
partition p holds rows (2p, 2p+1) of a 256-row block (4KB-contiguous
DRAM runs -> fat DMA packets). Block c = 2t+j has row(p) = 256t+2p+j;
labels arrive in a matching [8, 128] layout and are PE-transposed to
[128, 8]. Tile 0 is loaded as two half-tiles per tensor so block 0's
compute can start ~1.3 us earlier. The final 128-partition partial SSE
is reduced to [1,1] with a ones-matmul; host sums the 8 per-core
scalars.
"""

import numpy as np

import concourse.bacc as bacc
import concourse.bass as bass
import concourse.tile as tile
from concourse import mybir
from concourse.bass_utils import run_bass_kernel_spmd
from concourse.masks import make_identity
from concourse.vector_clock import ScopedClock


class _LeanTileContext(tile.TileContext):
    """TileContext with a minimal kernel epilogue.

    The stock epilogue is drain + all-engine butterfly + semaphore
    clear + second butterfly. For this single-shot kernel we only need
    the drain (all DMA queues complete, so the output is in DRAM before
    the NEFF retires); engines may retire their streams independently."""

    def _drain_and_barrier(self, tick_clock, wait_clock):
        drain_inst = self.nc.sync.drain()
        wait_clock.add_sem_waits(
            drain_inst.ins, ScopedClock({None: tick_clock.global_clock})
        )
        popped = self.nc._tile_sem_poison_stack.pop()
        assert popped is self._sem_poison


N, D = 8192, 1024
N_CORES = 8
ROWS = N // N_CORES  # rows per core
P = 128  # SBUF partitions
RPT = 2 * P  # rows per tile (2 per partition)
NTILES = ROWS // RPT  # 4
NBLK = 2 * NTILES  # 128-row blocks (tile t, half j -> c = 2t+j)
KCH = 8  # fold chunks per 1024-col product
PE_WARM = 8  # FD-512 warmup matmuls to ramp the PE p-state
PE_FILL = 1  # FD-256 filler matmuls per block to hold the p-state
PAIR_FILL = 2  # FD-512 fillers after each pair (bridge inter-tile DMA gap)

_cache = {}


def _build():
    nc = bacc.Bacc("TRN2", target_bir_lowering=False, debug=False)

    f32 = mybir.dt.float32
    f16 = mybir.dt.float16
    a = nc.dram_tensor("a", [ROWS, D], f16, kind="ExternalInput")
    b = nc.dram_tensor("b", [ROWS, D], f16, kind="ExternalInput")
    lab = nc.dram_tensor("lab_t", [NBLK, P], f32, kind="ExternalInput")
    out = nc.dram_tensor("out", [1, 1], f32, kind="ExternalOutput")

    with _LeanTileContext(nc) as tc:
        with (
            tc.tile_pool(name="io", bufs=NTILES) as io_pool,
            tc.tile_pool(name="prod", bufs=4) as prod_pool,
            tc.tile_pool(name="sq", bufs=2) as sq_pool,
            tc.tile_pool(name="fold", bufs=3, space="PSUM") as fold_pool,
            tc.tile_pool(name="psa", bufs=1, space="PSUM") as psa_pool,
            tc.tile_pool(name="stats", bufs=1) as st_pool,
        ):
            # --- upfront DMA: all 8 data tiles queued immediately ------
            # Tile 0 goes as four half-tile DMAs so block 0's inputs
            # complete in ~half the time; later tiles as single DMAs.
            # DMA descriptors cost ~155 ns each regardless of size, so
            # 4KB-per-partition runs (the full [P, 2048] tile) are the
            # smallest unit that sustains full bandwidth — never split.
            ats, bts = [], []
            lab_sb = st_pool.tile([NBLK, P], f32)
            for t in range(NTILES):
                at = io_pool.tile([P, 2 * D], f16, tag="a")
                bt = io_pool.tile([P, 2 * D], f16, tag="b")
                base = t * RPT * D
                # Split descriptor programming across both HWDGE engines
                # (Sync: a-tiles, ScalarE: b-tiles+labels). One engine
                # doing all 9 programs serializes ~6 us of DIRECT2D and
                # consumers' engine-clock waits release only at batch end.
                a_src = bass.AP(tensor=a, offset=base, ap=[[2 * D, P], [1, 2 * D]])
                b_src = bass.AP(tensor=b, offset=base, ap=[[2 * D, P], [1, 2 * D]])
                if t == 0:
                    # Labels (8 descriptors) first on the ACT ring: if
                    # issued later they stall on ring capacity and block
                    # ScalarE's in-order stream (squares started ~3.5us
                    # late without this).
                    nc.scalar.dma_start(out=lab_sb, in_=lab[:, :])
                nc.sync.dma_start(out=at, in_=a_src)
                nc.scalar.dma_start(out=bt, in_=b_src)
                ats.append(at)
                bts.append(bt)

            # --- constants -------------------------------------------
            na = st_pool.tile([P, NBLK], f32)
            # dot_c / nb_c interleaved: col 2c = dot_c, col 2c+1 = nb_c
            # (the per-pair segmented reduce writes 4 columns in one op).
            stats_db = st_pool.tile([P, 2 * NBLK], f32)

            ones = st_pool.tile([P, 1], f32)
            nc.vector.memset(ones, 1.0)
            # Warm the Sqrt activation table while DMA ramps up.
            warm = st_pool.tile([P, 1], f32)
            nc.scalar.sqrt(warm, ones)

            id8 = st_pool.tile([NBLK, NBLK], f32)
            make_identity(nc, id8)
            labt = psa_pool.tile([P, NBLK], f32)
            nc.tensor.transpose(labt, lab_sb, id8)

            id128 = st_pool.tile([P, P], f16)
            make_identity(nc, id128)
            wsrc = st_pool.tile([P, 512], f16)
            nc.vector.memset(wsrc, 0.0)

            # PE warmup: big moving-operand matmuls keep the PE busy
            # through its ~3us p-state ramp while the data DMA streams.
            wpsum = psa_pool.tile([P, 512], f32, tag="warm")
            for w in range(PE_WARM):
                nc.tensor.matmul(wpsum, id128, wsrc[:, :])

            # --- main loop: 8 blocks of 128 rows, paired for PSUM -----
            # Pair g holds slots (dot_2g, nb_2g, dot_2g+1, nb_2g+1) in
            # one PSUM bank; nb_7 goes to ScalarE so the last pair has
            # 3 slots and the last fold finishes sooner.
            fps = None
            for c in range(NBLK):
                t, j = divmod(c, 2)
                g, h = divmod(c, 2)  # pair index, slot half
                asl = ats[t][:, j * D : (j + 1) * D]
                bsl = bts[t][:, j * D : (j + 1) * D]

                # ScalarE: na_c = sum a^2 (nb_7 too).
                sa = sq_pool.tile([P, D], f16, tag="sq")
                nc.scalar.activation(
                    out=sa,
                    in_=asl,
                    func=mybir.ActivationFunctionType.Square,
                    accum_out=na[:, c : c + 1],
                )
                if c == NBLK - 1:
                    sb7 = sq_pool.tile([P, D], f16, tag="sq")
                    nc.scalar.activation(
                        out=sb7,
                        in_=bsl,
                        func=mybir.ActivationFunctionType.Square,
                        accum_out=stats_db[:, 2 * c + 1 : 2 * c + 2],
                    )

                # VectorE products (2x_1P fp16): ab (and bb except c=7)
                # in halves of one scratch tile so the PE fold reads
                # both with a single 3D AP.
                nslot = 1 if c == NBLK - 1 else 2
                pt = prod_pool.tile([P, 2 * D], f16, tag="p")
                nc.vector.tensor_mul(pt[:, 0:D], asl, bsl)
                if nslot == 2:
                    nc.vector.tensor_mul(pt[:, D : 2 * D], bsl, bsl)

                # TensorE: fold [P, nslot*1024] -> PSUM slots
                # [P, nslot, 128] of the pair's bank.
                if h == 0:
                    fps = fold_pool.tile([P, 4 if g < 3 else 3, P], f32)
                pt4 = pt[:, 0 : nslot * D].rearrange(
                    "p (s k c) -> p s k c", s=nslot, k=KCH, c=P
                )
                for k in range(KCH):
                    nc.tensor.matmul(
                        fps[:, 2 * h : 2 * h + nslot, :],
                        id128,
                        pt4[:, :, k, :],
                        start=(k == 0),
                        stop=(k == KCH - 1),
                    )
                # Fillers: cheap matmuls right after each block's folds
                # absorb PE idle so the p-state doesn't decay. They read
                # this block's product so the scheduler can't hoist them.
                for w in range(PE_FILL):
                    nc.tensor.matmul(
                        wpsum[:, 0:256], id128, pt[:, 256 * w : 256 * (w + 1)]
                    )
                if h == 1 and g < 3:
                    # Bridge the inter-tile DMA gap (~1.1 us of PE work).
                    for w in range(PAIR_FILL):
                        nc.tensor.matmul(wpsum, id128, pt[:, 0:512])

                if h == 1 or c == NBLK - 1:
                    # VectorE: segmented reduce -> 4 (3) stat columns.
                    ns = 4 if g < 3 else 3
                    nc.vector.tensor_reduce(
                        out=stats_db[:, 4 * g : 4 * g + ns],
                        in_=fps,
                        axis=mybir.AxisListType.X,
                        op=mybir.AluOpType.add,
                    )

            # --- tail ------------------------------------------------
            # score = dot * rsqrt(na*nb); diff = score - label. Groups
            # 0-2 (blocks 0..5) batch into one pass once pair 2 lands
            # (off the critical path while tile 3 streams); group 3 is
            # the only tail work after the last block's stats.
            diff = st_pool.tile([P, 2 * NBLK], f32)  # cols 2c used
            for lo, hi in ((0, 6), (6, 8)):
                w = hi - lo
                cols = slice(2 * lo, 2 * hi, 2)
                nbv = stats_db[:, 2 * lo + 1 : 2 * hi : 2]
                nav = na[:, lo:hi]
                dv = stats_db[:, 2 * lo : 2 * hi : 2]
                pr = st_pool.tile([P, w], f32, tag=f"pr{lo}")
                nc.vector.tensor_mul(pr, nav, nbv)
                nc.scalar.sqrt(pr, pr)
                rs = st_pool.tile([P, w], f32, tag=f"rs{lo}")
                nc.vector.reciprocal(rs, pr)
                sc = st_pool.tile([P, w], f32, tag=f"sc{lo}")
                nc.vector.tensor_mul(sc, dv, rs)
                nc.vector.tensor_sub(diff[:, cols], sc, labt[:, lo:hi])

            sqd = st_pool.tile([P, NBLK], f32)
            partial = st_pool.tile([P, 1], f32)
            nc.vector.scalar_tensor_tensor(
                out=sqd,
                in0=diff[:, 0 : 2 * NBLK : 2],
                scalar=1.0,
                in1=diff[:, 0 : 2 * NBLK : 2],
                op0=mybir.AluOpType.mult,
                op1=mybir.AluOpType.mult,
                accum_out=partial,
            )
            # Reduce 128 partitions -> [1,1] so the output DMA is one
            # descriptor instead of 128.
            total_ps = psa_pool.tile([1, 1], f32)
            nc.tensor.matmul(total_ps, partial, ones)
            res_sb = st_pool.tile([1, 1], f32)
            nc.scalar.copy(res_sb, total_ps)
            nc.sync.dma_start(out=out[:, :], in_=res_sb)

    nc.compile()
    return nc


def _label_perm(lab_core):
    """[ROWS] -> [NBLK, P] so that PE-transpose yields labt[p, c] =
    labels[256*(c//2) + 2p + (c%2)], matching the stats layout."""
    return np.ascontiguousarray(
        lab_core.reshape(NTILES, P, 2).transpose(0, 2, 1).reshape(NBLK, P)
    )


def kernel(issues_1_geb, issues_2_geb, labels):
    if "nc" not in _cache:
        _cache["nc"] = _build()
    nc = _cache["nc"]

    a16 = np.ascontiguousarray(issues_1_geb, dtype=np.float16)
    b16 = np.ascontiguousarray(issues_2_geb, dtype=np.float16)
    lab = np.ascontiguousarray(labels, dtype=np.float32)

    in_maps = []
    for c in range(N_CORES):
        sl = slice(c * ROWS, (c + 1) * ROWS)
        in_maps.append(
            {
                "a": np.ascontiguousarray(a16[sl]),
                "b": np.ascontiguousarray(b16[sl]),
                "lab_t": _label_perm(lab[sl]),
            }
        )

    res = run_bass_kernel_spmd(nc, in_maps, core_ids=list(range(N_CORES)))
    total = np.float64(0.0)
    for r in res.results:
        total += np.float64(r["out"].sum(dtype=np.float64))
    return np.array(total / N, dtype=np.float32)

